# revision 16
# baseline (speedup 1.0000x reference)
"""LIF spike kernel for Trainium2 (Bass/Tile), data-parallel over 8 NeuronCores.

Problem: x [32, 8, 128, 32, 32] fp32 -> spikes [32, 8, 128, 32, 32] fp32
    mem_t = mem_{t-1} * 0.25 + x_t ; spike = (mem >= 0.5) ; mem *= (1 - spike)

Sharding: batch dim (32) split 4-per-core across 8 cores; no cross-core comm.

Per-core device program (host pre-transposes the core's slab to [T, C, B*HW]
so every DMA is a fully contiguous HBM stream; host undoes it after):
  - layout: partitions = channel dim C=128, free = (b, h*w) = 4096
  - membrane update is ONE fused custom DVE op per step (vs 2 stock ops):
        u_t = select(u_{t-1} < 0.5, u_{t-1}, 0) * TAU + x_t
    (mask-mult and *0.25 are exact in fp32; single rounding on the add —
    bitwise identical to the jax fp32 reference)
  - spike on the ACT engine: y_t = Sign(u_t - 0.5) -> uint8 {255/0, 0, 1};
    host decodes spike := (y == 1). ACT table is prewarmed before the loop.
  - x loads batched (STEPS_PER_LOAD steps per dma_start) on the SP HWDGE
    ring; y stores ride the ACT ring right after their Sign (program order,
    no extra sem wait).
"""

import os
import numpy as np

B, T, C, H, W = 32, 8, 128, 32, 32
HW = H * W
N_CORES = 8
BPC = B // N_CORES  # batches per core
FREE = BPC * HW  # 4096
TAU = 0.25
THRESH = 0.5

_nc_cache = {}
LAST_RESULTS = None


def _register_lif_op():
    """Register the fused LIF membrane-update op with the custom-DVE table
    (runtime equivalent of the documented two-edit dve_ops.py append)."""
    import concourse.dve_ops as dv
    from concourse.dve_spec import Spec, Src0, Src1, C0, C1, Zero, select

    for op in dv.OPS:
        if op.name == "LIF_FUSED_ANT":
            return op
    op = dv.DveOp(
        "LIF_FUSED_ANT",
        Spec(
            body=select(Src0 < C0, Src0, Zero) * C1 + Src1,
            reference=lambda in0, in1, s0, s1, imm2: (
                np.where(in0 < s0, in0, np.float32(0.0)) * np.float32(s1) + in1
            ).astype(np.float32),
        ),
        subdim=False,
        uops_sha={"v3": "dc49afe33bac4c9a", "v4": "05a48bcb07e07a04"},
    )
    dv.OPS.append(op)
    dv._SUB_OPCODE_FOR_NAME[op.name] = dv._CUSTOM_DVE_ROW_BASE + len(dv.OPS) - 1
    dv.CUSTOM_DVE_SPECS[op.name] = op.spec
    assert max(dv._SUB_OPCODE_FOR_NAME.values()) < 0x20
    return op


def build_bass_fused(reps=1, steps_per_load=2, nsplit=2, xp_bufs=None):
    """Per-core program on the [T, C, FREE] layout.

    steps_per_load: time steps batched into one input dma_start (bigger =
    better DMA efficiency, coarser pipeline granularity).
    nsplit: free-dim chunks per step for DVE/ACT (finer = shorter one-shot
    tail; throughput cost is the ~58-cycle per-op overhead).
    """
    import concourse.bacc as bacc
    import concourse.mybir as mybir
    from concourse.tile import TileContext

    lif_op = _register_lif_op()

    f32 = mybir.dt.float32
    u8 = mybir.dt.uint8

    assert T % steps_per_load == 0
    n_groups = T // steps_per_load
    if xp_bufs is None:
        # keep the whole-rep input (128 KiB/partition) outstanding
        xp_bufs = max(2, n_groups)
    assert FREE % nsplit == 0
    CH = FREE // nsplit

    nc = bacc.Bacc("TRN2", target_bir_lowering=False)
    x = nc.dram_tensor("x", [T, C, FREE], f32, kind="ExternalInput")
    y = nc.dram_tensor("y", [T, C, FREE], u8, kind="ExternalOutput")

    with TileContext(nc) as tc:
        with (
            tc.tile_pool(name="xp", bufs=xp_bufs) as xp,
            tc.tile_pool(name="up", bufs=3) as up,
            tc.tile_pool(name="yp", bufs=3) as yp,
            tc.tile_pool(name="cp", bufs=1) as cp,
        ):
            neg_thresh = cp.tile([C, 1], f32)
            nc.vector.memset(neg_thresh[:], -THRESH)
            warm = cp.tile([C, 1], u8)
            # prewarm the Sign table so the ~2.7us ACT_TABLE_LOAD overlaps
            # the first x load instead of sitting on the critical path
            nc.scalar.activation(
                warm[:], neg_thresh[:], mybir.ActivationFunctionType.Sign
            )
            for _rep in range(reps):
                xts = []
                for g in range(n_groups):
                    xt = xp.tile([C, steps_per_load, FREE], f32, tag="xt")
                    nc.sync.dma_start(
                        xt[:],
                        x[
                            g * steps_per_load : (g + 1) * steps_per_load
                        ].rearrange("t c w -> c t w"),
                    )
                    xts.append(xt)
                u_prev = None
                for t in range(T):
                    g, o = divmod(t, steps_per_load)
                    xsl = xts[g][:, o, :]
                    if t == 0:
                        u = xsl
                    else:
                        ut = up.tile([C, FREE], f32)
                        for j in range(nsplit):
                            s = slice(j * CH, (j + 1) * CH)
                            nc.vector._custom_dve(
                                lif_op,
                                out=ut[:, s],
                                in0=u_prev[:, s],
                                in1=xsl[:, s],
                                s0=THRESH,
                                s1=TAU,
                            )
                        u = ut
                    yt = yp.tile([C, FREE], u8)
                    for j in range(nsplit):
                        s = slice(j * CH, (j + 1) * CH)
                        nc.scalar.activation(
                            yt[:, s],
                            u[:, s],
                            mybir.ActivationFunctionType.Sign,
                            bias=neg_thresh[:],
                        )
                    nc.scalar.dma_start(y[t], yt[:])
                    u_prev = u
    nc.compile()
    return nc


def build_bass_fused2(reps=1, nsplit=2, store_split=2, xp_bufs=None):
    """Fully chunked pipeline: each step's x is loaded as `nsplit` separate
    chunk tiles (per-chunk dependency granularity), the fused DVE op and the
    ACT Sign run per chunk, and stores go out per `store_split` sub-chunk.
    Minimizes the post-last-load tail of a one-shot run."""
    import concourse.bacc as bacc
    import concourse.mybir as mybir
    from concourse.tile import TileContext

    lif_op = _register_lif_op()

    f32 = mybir.dt.float32
    u8 = mybir.dt.uint8

    assert FREE % nsplit == 0
    CH = FREE // nsplit
    if xp_bufs is None:
        xp_bufs = 2 * T * max(1, nsplit // 2)  # whole rep in flight

    nc = bacc.Bacc("TRN2", target_bir_lowering=False)
    x = nc.dram_tensor("x", [T, C, FREE], f32, kind="ExternalInput")
    y = nc.dram_tensor("y", [T, C, FREE], u8, kind="ExternalOutput")

    with TileContext(nc) as tc:
        with (
            tc.tile_pool(name="xp", bufs=xp_bufs) as xp,
            tc.tile_pool(name="up", bufs=3) as up,
            tc.tile_pool(name="yp", bufs=3) as yp,
            tc.tile_pool(name="cp", bufs=1) as cp,
        ):
            neg_thresh = cp.tile([C, 1], f32)
            nc.vector.memset(neg_thresh[:], -THRESH)
            warm = cp.tile([C, 1], u8)
            nc.scalar.activation(
                warm[:], neg_thresh[:], mybir.ActivationFunctionType.Sign
            )
            for _rep in range(reps):
                xts = []
                for t in range(T):
                    for j in range(nsplit):
                        xt = xp.tile([C, CH], f32, tag="xt")
                        nc.sync.dma_start(
                            xt[:], x[t, :, j * CH : (j + 1) * CH]
                        )
                        xts.append(xt)
                u_prev = None
                for t in range(T):
                    if t == 0:
                        u = xts[:nsplit]
                    else:
                        ut = up.tile([C, FREE], f32)
                        for j in range(nsplit):
                            s = slice(j * CH, (j + 1) * CH)
                            nc.vector._custom_dve(
                                lif_op,
                                out=ut[:, s],
                                in0=u_prev[j] if t == 1 else u_prev[:, s],
                                in1=xts[t * nsplit + j][:],
                                s0=THRESH,
                                s1=TAU,
                            )
                        u = ut
                    yt = yp.tile([C, FREE], u8)
                    for j in range(nsplit):
                        s = slice(j * CH, (j + 1) * CH)
                        nc.scalar.activation(
                            yt[:, s],
                            u[j][:] if t == 0 else u[:, s],
                            mybir.ActivationFunctionType.Sign,
                            bias=neg_thresh[:],
                        )
                        if store_split == nsplit:
                            # store chunk right behind its Sign on the ACT
                            # ring: program order, minimal tail
                            nc.scalar.dma_start(y[t, :, s], yt[:, s])
                    if store_split != nsplit:
                        for m in range(store_split):
                            sm = slice(
                                m * (FREE // store_split),
                                (m + 1) * (FREE // store_split),
                            )
                            nc.scalar.dma_start(y[t, :, sm], yt[:, sm])
                    u_prev = u
    nc.compile()
    return nc


def build_bass_fused3(reps=1, nsplit=2, tail_split=4, tail_steps=2, xp_bufs=None):
    """Hybrid pipeline: one 2.1MB load per step for the body steps
    (HW-efficient transfer size); the last `tail_steps` steps' x loaded as
    `tail_split` chunk tiles each so the recurrence tail drains right behind
    the DMA stream; compute/stores chunked (`nsplit` body, `tail_split`
    tail); each store issued on the ACT ring directly after its Sign."""
    import concourse.bacc as bacc
    import concourse.mybir as mybir
    from concourse.tile import TileContext

    lif_op = _register_lif_op()

    f32 = mybir.dt.float32
    u8 = mybir.dt.uint8

    assert FREE % nsplit == 0 and FREE % tail_split == 0
    n_body = T - tail_steps
    if xp_bufs is None:
        xp_bufs = n_body  # body tiles of one rep (SBUF budget cap)

    nc = bacc.Bacc("TRN2", target_bir_lowering=False)
    x = nc.dram_tensor("x", [T, C, FREE], f32, kind="ExternalInput")
    y = nc.dram_tensor("y", [T, C, FREE], u8, kind="ExternalOutput")

    with TileContext(nc) as tc:
        with (
            tc.tile_pool(name="xp", bufs=xp_bufs) as xp,
            tc.tile_pool(name="tp", bufs=tail_steps * tail_split) as tp,
            tc.tile_pool(name="up", bufs=3) as up,
            tc.tile_pool(name="yp", bufs=3) as yp,
            tc.tile_pool(name="cp", bufs=1) as cp,
        ):
            neg_thresh = cp.tile([C, 1], f32)
            nc.vector.memset(neg_thresh[:], -THRESH)
            warm = cp.tile([C, 1], u8)
            nc.scalar.activation(
                warm[:], neg_thresh[:], mybir.ActivationFunctionType.Sign
            )
            TCH = FREE // tail_split
            for _rep in range(reps):
                xts = {}
                for t in range(n_body):
                    xt = xp.tile([C, FREE], f32, tag="xt")
                    nc.sync.dma_start(xt[:], x[t])
                    xts[t] = xt
                tails = {}
                for t in range(n_body, T):
                    for j in range(tail_split):
                        xt = tp.tile([C, TCH], f32, tag="xtail")
                        nc.sync.dma_start(
                            xt[:], x[t, :, j * TCH : (j + 1) * TCH]
                        )
                        tails[(t, j)] = xt
                u_prev = None
                for t in range(T):
                    tail = t >= n_body
                    ns = tail_split if tail else nsplit
                    CHt = FREE // ns
                    if t == 0:
                        u = xts[0]
                    else:
                        ut = up.tile([C, FREE], f32)
                        for j in range(ns):
                            s = slice(j * CHt, (j + 1) * CHt)
                            nc.vector._custom_dve(
                                lif_op,
                                out=ut[:, s],
                                in0=u_prev[:, s],
                                in1=tails[(t, j)][:] if tail else xts[t][:, s],
                                s0=THRESH,
                                s1=TAU,
                            )
                        u = ut
                    yt = yp.tile([C, FREE], u8)
                    for j in range(ns):
                        s = slice(j * CHt, (j + 1) * CHt)
                        nc.scalar.activation(
                            yt[:, s],
                            u[:, s],
                            mybir.ActivationFunctionType.Sign,
                            bias=neg_thresh[:],
                        )
                        nc.scalar.dma_start(y[t, :, s], yt[:, s])
                    u_prev = u
    nc.compile()
    return nc


def build_bass_fused4(reps=1, xp_bufs=None):
    """Natural-layout chunked pipeline: I/O tensors keep the host layout
    [BPC, T, C, HW]; the chunk unit is one batch element, so every load
    (x[b,t], 512KB) and store (y[b,t], 128KB) is fully contiguous in DRAM
    and the host does no transposes at all. Compute is per-chunk: fused LIF
    DVE op + ACT Sign + store right behind it on the ACT ring."""
    import concourse.bacc as bacc
    import concourse.mybir as mybir
    from concourse.tile import TileContext

    lif_op = _register_lif_op()

    f32 = mybir.dt.float32
    u8 = mybir.dt.uint8

    if xp_bufs is None:
        xp_bufs = 2 * T  # half a rep of x chunks in flight

    nc = bacc.Bacc("TRN2", target_bir_lowering=False)
    x = nc.dram_tensor("x", [BPC, T, C, HW], f32, kind="ExternalInput")
    y = nc.dram_tensor("y", [BPC, T, C, HW], u8, kind="ExternalOutput")

    with TileContext(nc) as tc:
        with (
            tc.tile_pool(name="xp", bufs=xp_bufs) as xp,
            tc.tile_pool(name="up", bufs=3) as up,
            tc.tile_pool(name="yp", bufs=3) as yp,
            tc.tile_pool(name="cp", bufs=1) as cp,
        ):
            neg_thresh = cp.tile([C, 1], f32)
            nc.vector.memset(neg_thresh[:], -THRESH)
            warm = cp.tile([C, 1], u8)
            nc.scalar.activation(
                warm[:], neg_thresh[:], mybir.ActivationFunctionType.Sign
            )
            for _rep in range(reps):
                xts = {}
                for t in range(T):
                    for b in range(BPC):
                        xt = xp.tile([C, HW], f32, tag="xt")
                        nc.sync.dma_start(xt[:], x[b, t])
                        xts[(t, b)] = xt
                u_prev = None
                for t in range(T):
                    if t == 0:
                        u = [xts[(0, b)] for b in range(BPC)]
                    else:
                        ut = up.tile([C, BPC, HW], f32)
                        for b in range(BPC):
                            nc.vector._custom_dve(
                                lif_op,
                                out=ut[:, b, :],
                                in0=u_prev[b][:] if t == 1 else u_prev[:, b, :],
                                in1=xts[(t, b)][:],
                                s0=THRESH,
                                s1=TAU,
                            )
                        u = ut
                    yt = yp.tile([C, BPC, HW], u8)
                    for b in range(BPC):
                        nc.scalar.activation(
                            yt[:, b, :],
                            u[b][:] if t == 0 else u[:, b, :],
                            mybir.ActivationFunctionType.Sign,
                            bias=neg_thresh[:],
                        )
                        nc.scalar.dma_start(y[b, t], yt[:, b, :])
                    u_prev = u
    nc.compile()
    return nc


# ---- legacy variant (previous session's kernel) for A/B ---------------------
def build_bass(free_w=HW, use_act=True, reps=1):
    import concourse.bacc as bacc
    import concourse.mybir as mybir
    from concourse.tile import TileContext

    f32 = mybir.dt.float32
    u8 = mybir.dt.uint8
    Alu = mybir.AluOpType

    nc = bacc.Bacc("TRN2", target_bir_lowering=False)
    x = nc.dram_tensor("x", [BPC, T, C, free_w], f32, kind="ExternalInput")
    y = nc.dram_tensor("y", [BPC, T, C, free_w], u8, kind="ExternalOutput")

    with TileContext(nc) as tc:
        with (
            tc.tile_pool(name="xp", bufs=6) as xp,
            tc.tile_pool(name="up", bufs=2) as up,
            tc.tile_pool(name="rp", bufs=2) as rp,
            tc.tile_pool(name="yp", bufs=3) as yp,
            tc.tile_pool(name="cp", bufs=1) as cp,
        ):
            neg_thresh = None
            if use_act:
                neg_thresh = cp.tile([C, 1], f32)
                nc.vector.memset(neg_thresh[:], -THRESH)
            for _rep in range(reps):
                r = None
                for t in range(T):
                    xt = xp.tile([C, BPC, free_w], f32)
                    nc.sync.dma_start(xt[:], x[:, t, :, :].rearrange("b c w -> c b w"))
                    if t == 0:
                        u = xt
                    else:
                        u = up.tile([C, BPC, free_w], f32)
                        nc.vector.scalar_tensor_tensor(
                            u[:], r[:], TAU, xt[:], Alu.mult, Alu.add
                        )
                    yt = yp.tile([C, BPC, free_w], u8)
                    if use_act:
                        nc.scalar.activation(
                            yt[:],
                            u[:],
                            mybir.ActivationFunctionType.Sign,
                            bias=neg_thresh[:],
                        )
                    else:
                        nc.vector.tensor_scalar(yt[:], u[:], THRESH, None, Alu.is_ge)
                    if t < T - 1:
                        rn = rp.tile([C, BPC, free_w], f32)
                        nc.vector.scalar_tensor_tensor(
                            rn[:], u[:], THRESH, u[:], Alu.is_lt, Alu.mult
                        )
                        r = rn
                    nc.scalar.dma_start(
                        y[:, t, :, :].rearrange("b c w -> c b w"), yt[:]
                    )
    nc.compile()
    return nc


# Default device-program config. "ck4s2": fully chunked pipeline, 4 chunks
# per step (1KB-line loads of 0.52MB), stores per 2 chunks. Overridable via
# LIF_CFG for A/B testing; see build_cfg for the grammar.
DEFAULT_CFG = "ck4s2"


def build_cfg(cfg, reps=1):
    if cfg == "act":
        return build_bass(HW, use_act=True, reps=reps)
    if cfg == "nat":
        return build_bass_fused4(reps=reps)
    if cfg.startswith("ck"):
        ns, ss = cfg[2:].split("s")
        return build_bass_fused2(reps=reps, nsplit=int(ns), store_split=int(ss))
    if cfg.startswith("f3"):
        ns, rest = cfg[3:].split("t")
        ts, tst = (rest.split("s") + ["1"])[:2] if "s" in rest else (rest, "1")
        return build_bass_fused3(
            reps=reps, nsplit=int(ns), tail_split=int(ts), tail_steps=int(tst)
        )
    spl, ns = int(cfg[3]), int(cfg[6])  # spl<k>ns<j>
    return build_bass_fused(reps=reps, steps_per_load=spl, nsplit=ns)


def _get_nc():
    cfg = os.environ.get("LIF_CFG", DEFAULT_CFG)
    if cfg not in _nc_cache:
        _nc_cache[cfg] = build_cfg(cfg)
    return _nc_cache[cfg]


def kernel(x):
    global LAST_RESULTS
    from concourse import bass_utils

    assert x.shape == (B, T, C, H, W) and x.dtype == np.float32
    cfg = os.environ.get("LIF_CFG", DEFAULT_CFG)
    nc = _get_nc()
    if cfg in ("act", "nat"):
        xr = x.reshape(B, T, C, HW)
        in_maps = [{"x": xr[i * BPC : (i + 1) * BPC]} for i in range(N_CORES)]
    else:
        # per core i: x[4i:4i+4] as [T, C, BPC*HW] contiguous
        xs = np.ascontiguousarray(
            x.reshape(N_CORES, BPC, T, C, HW).transpose(0, 2, 3, 1, 4)
        ).reshape(N_CORES, T, C, FREE)
        in_maps = [{"x": xs[i]} for i in range(N_CORES)]
    res = bass_utils.run_bass_kernel_spmd(
        nc,
        in_maps,
        core_ids=list(range(N_CORES)),
        trace=bool(int(os.environ.get("LIF_TRACE", "0"))),
    )
    LAST_RESULTS = res
    out = np.empty((B, T, C, HW), dtype=np.float32)
    for i in range(N_CORES):
        yi = res.results[i]["y"]
        if cfg in ("act", "nat"):
            out[i * BPC : (i + 1) * BPC] = yi == 1
        else:
            # yi [T, C, FREE]; Sign lands {255/0, 0, 1} in uint8; spike==1
            sp = yi.reshape(T, C, BPC, HW).transpose(2, 0, 1, 3)
            out[i * BPC : (i + 1) * BPC] = sp == 1
    return out.reshape(B, T, C, H, W)


# revision 17
# speedup vs baseline: 1.0156x; 1.0156x over previous
"""LIF spike kernel for Trainium2 (Bass/Tile), data-parallel over 8 NeuronCores.

Problem: x [32, 8, 128, 32, 32] fp32 -> spikes [32, 8, 128, 32, 32] fp32
    mem_t = mem_{t-1} * 0.25 + x_t ; spike = (mem >= 0.5) ; mem *= (1 - spike)

Sharding: batch dim (32) split 4-per-core across 8 cores; no cross-core comm.

Per-core device program (host pre-transposes the core's slab to [T, C, B*HW]
so every DMA is a fully contiguous HBM stream; host undoes it after):
  - layout: partitions = channel dim C=128, free = (b, h*w) = 4096
  - membrane update is ONE fused custom DVE op per step (vs 2 stock ops):
        u_t = select(u_{t-1} < 0.5, u_{t-1}, 0) * TAU + x_t
    (mask-mult and *0.25 are exact in fp32; single rounding on the add —
    bitwise identical to the jax fp32 reference)
  - spike on the ACT engine: y_t = Sign(u_t - 0.5) -> uint8 {255/0, 0, 1};
    host decodes spike := (y == 1). ACT table is prewarmed before the loop.
  - x loads batched (STEPS_PER_LOAD steps per dma_start) on the SP HWDGE
    ring; y stores ride the ACT ring right after their Sign (program order,
    no extra sem wait).
"""

import os
import numpy as np

B, T, C, H, W = 32, 8, 128, 32, 32
HW = H * W
N_CORES = 8
BPC = B // N_CORES  # batches per core
FREE = BPC * HW  # 4096
TAU = 0.25
THRESH = 0.5

_nc_cache = {}
LAST_RESULTS = None


def _register_lif_op():
    """Register the fused LIF membrane-update op with the custom-DVE table
    (runtime equivalent of the documented two-edit dve_ops.py append)."""
    import concourse.dve_ops as dv
    from concourse.dve_spec import Spec, Src0, Src1, C0, C1, Zero, select

    for op in dv.OPS:
        if op.name == "LIF_FUSED_ANT":
            return op
    op = dv.DveOp(
        "LIF_FUSED_ANT",
        Spec(
            body=select(Src0 < C0, Src0, Zero) * C1 + Src1,
            reference=lambda in0, in1, s0, s1, imm2: (
                np.where(in0 < s0, in0, np.float32(0.0)) * np.float32(s1) + in1
            ).astype(np.float32),
        ),
        subdim=False,
        uops_sha={"v3": "dc49afe33bac4c9a", "v4": "05a48bcb07e07a04"},
    )
    dv.OPS.append(op)
    dv._SUB_OPCODE_FOR_NAME[op.name] = dv._CUSTOM_DVE_ROW_BASE + len(dv.OPS) - 1
    dv.CUSTOM_DVE_SPECS[op.name] = op.spec
    assert max(dv._SUB_OPCODE_FOR_NAME.values()) < 0x20
    return op


def build_bass_fused(reps=1, steps_per_load=2, nsplit=2, xp_bufs=None):
    """Per-core program on the [T, C, FREE] layout.

    steps_per_load: time steps batched into one input dma_start (bigger =
    better DMA efficiency, coarser pipeline granularity).
    nsplit: free-dim chunks per step for DVE/ACT (finer = shorter one-shot
    tail; throughput cost is the ~58-cycle per-op overhead).
    """
    import concourse.bacc as bacc
    import concourse.mybir as mybir
    from concourse.tile import TileContext

    lif_op = _register_lif_op()

    f32 = mybir.dt.float32
    u8 = mybir.dt.uint8

    assert T % steps_per_load == 0
    n_groups = T // steps_per_load
    if xp_bufs is None:
        # keep the whole-rep input (128 KiB/partition) outstanding
        xp_bufs = max(2, n_groups)
    assert FREE % nsplit == 0
    CH = FREE // nsplit

    nc = bacc.Bacc("TRN2", target_bir_lowering=False)
    x = nc.dram_tensor("x", [T, C, FREE], f32, kind="ExternalInput")
    y = nc.dram_tensor("y", [T, C, FREE], u8, kind="ExternalOutput")

    with TileContext(nc) as tc:
        with (
            tc.tile_pool(name="xp", bufs=xp_bufs) as xp,
            tc.tile_pool(name="up", bufs=3) as up,
            tc.tile_pool(name="yp", bufs=3) as yp,
            tc.tile_pool(name="cp", bufs=1) as cp,
        ):
            neg_thresh = cp.tile([C, 1], f32)
            nc.vector.memset(neg_thresh[:], -THRESH)
            warm = cp.tile([C, 1], u8)
            # prewarm the Sign table so the ~2.7us ACT_TABLE_LOAD overlaps
            # the first x load instead of sitting on the critical path
            nc.scalar.activation(
                warm[:], neg_thresh[:], mybir.ActivationFunctionType.Sign
            )
            for _rep in range(reps):
                xts = []
                for g in range(n_groups):
                    xt = xp.tile([C, steps_per_load, FREE], f32, tag="xt")
                    nc.sync.dma_start(
                        xt[:],
                        x[
                            g * steps_per_load : (g + 1) * steps_per_load
                        ].rearrange("t c w -> c t w"),
                    )
                    xts.append(xt)
                u_prev = None
                for t in range(T):
                    g, o = divmod(t, steps_per_load)
                    xsl = xts[g][:, o, :]
                    if t == 0:
                        u = xsl
                    else:
                        ut = up.tile([C, FREE], f32)
                        for j in range(nsplit):
                            s = slice(j * CH, (j + 1) * CH)
                            nc.vector._custom_dve(
                                lif_op,
                                out=ut[:, s],
                                in0=u_prev[:, s],
                                in1=xsl[:, s],
                                s0=THRESH,
                                s1=TAU,
                            )
                        u = ut
                    yt = yp.tile([C, FREE], u8)
                    for j in range(nsplit):
                        s = slice(j * CH, (j + 1) * CH)
                        nc.scalar.activation(
                            yt[:, s],
                            u[:, s],
                            mybir.ActivationFunctionType.Sign,
                            bias=neg_thresh[:],
                        )
                    nc.scalar.dma_start(y[t], yt[:])
                    u_prev = u
    nc.compile()
    return nc


def build_bass_fused2(reps=1, nsplit=2, store_split=2, xp_bufs=None):
    """Fully chunked pipeline: each step's x is loaded as `nsplit` separate
    chunk tiles (per-chunk dependency granularity), the fused DVE op and the
    ACT Sign run per chunk, and stores go out per `store_split` sub-chunk.
    Minimizes the post-last-load tail of a one-shot run."""
    import concourse.bacc as bacc
    import concourse.mybir as mybir
    from concourse.tile import TileContext

    lif_op = _register_lif_op()

    f32 = mybir.dt.float32
    u8 = mybir.dt.uint8

    assert FREE % nsplit == 0
    CH = FREE // nsplit
    if xp_bufs is None:
        xp_bufs = 2 * T * max(1, nsplit // 2)  # whole rep in flight

    nc = bacc.Bacc("TRN2", target_bir_lowering=False)
    x = nc.dram_tensor("x", [T, C, FREE], f32, kind="ExternalInput")
    y = nc.dram_tensor("y", [T, C, FREE], u8, kind="ExternalOutput")

    with TileContext(nc) as tc:
        with (
            tc.tile_pool(name="xp", bufs=xp_bufs) as xp,
            tc.tile_pool(name="up", bufs=3) as up,
            tc.tile_pool(name="yp", bufs=3) as yp,
            tc.tile_pool(name="cp", bufs=1) as cp,
        ):
            neg_thresh = cp.tile([C, 1], f32)
            nc.vector.memset(neg_thresh[:], -THRESH)
            warm = cp.tile([C, 1], u8)
            nc.scalar.activation(
                warm[:], neg_thresh[:], mybir.ActivationFunctionType.Sign
            )
            for _rep in range(reps):
                xts = []
                for t in range(T):
                    for j in range(nsplit):
                        xt = xp.tile([C, CH], f32, tag="xt")
                        nc.sync.dma_start(
                            xt[:], x[t, :, j * CH : (j + 1) * CH]
                        )
                        xts.append(xt)
                u_prev = None
                for t in range(T):
                    if t == 0:
                        u = xts[:nsplit]
                    else:
                        ut = up.tile([C, FREE], f32)
                        for j in range(nsplit):
                            s = slice(j * CH, (j + 1) * CH)
                            nc.vector._custom_dve(
                                lif_op,
                                out=ut[:, s],
                                in0=u_prev[j] if t == 1 else u_prev[:, s],
                                in1=xts[t * nsplit + j][:],
                                s0=THRESH,
                                s1=TAU,
                            )
                        u = ut
                    yt = yp.tile([C, FREE], u8)
                    for j in range(nsplit):
                        s = slice(j * CH, (j + 1) * CH)
                        nc.scalar.activation(
                            yt[:, s],
                            u[j][:] if t == 0 else u[:, s],
                            mybir.ActivationFunctionType.Sign,
                            bias=neg_thresh[:],
                        )
                        if store_split == nsplit:
                            # store chunk right behind its Sign on the ACT
                            # ring: program order, minimal tail
                            nc.scalar.dma_start(y[t, :, s], yt[:, s])
                    if store_split != nsplit:
                        for m in range(store_split):
                            sm = slice(
                                m * (FREE // store_split),
                                (m + 1) * (FREE // store_split),
                            )
                            nc.scalar.dma_start(y[t, :, sm], yt[:, sm])
                    u_prev = u
    nc.compile()
    return nc


def build_bass_fused3(reps=1, nsplit=2, tail_split=4, tail_steps=2, xp_bufs=None):
    """Hybrid pipeline: one 2.1MB load per step for the body steps
    (HW-efficient transfer size); the last `tail_steps` steps' x loaded as
    `tail_split` chunk tiles each so the recurrence tail drains right behind
    the DMA stream; compute/stores chunked (`nsplit` body, `tail_split`
    tail); each store issued on the ACT ring directly after its Sign."""
    import concourse.bacc as bacc
    import concourse.mybir as mybir
    from concourse.tile import TileContext

    lif_op = _register_lif_op()

    f32 = mybir.dt.float32
    u8 = mybir.dt.uint8

    assert FREE % nsplit == 0 and FREE % tail_split == 0
    n_body = T - tail_steps
    if xp_bufs is None:
        xp_bufs = n_body  # body tiles of one rep (SBUF budget cap)

    nc = bacc.Bacc("TRN2", target_bir_lowering=False)
    x = nc.dram_tensor("x", [T, C, FREE], f32, kind="ExternalInput")
    y = nc.dram_tensor("y", [T, C, FREE], u8, kind="ExternalOutput")

    with TileContext(nc) as tc:
        with (
            tc.tile_pool(name="xp", bufs=xp_bufs) as xp,
            tc.tile_pool(name="tp", bufs=tail_steps * tail_split) as tp,
            tc.tile_pool(name="up", bufs=3) as up,
            tc.tile_pool(name="yp", bufs=3) as yp,
            tc.tile_pool(name="cp", bufs=1) as cp,
        ):
            neg_thresh = cp.tile([C, 1], f32)
            nc.vector.memset(neg_thresh[:], -THRESH)
            warm = cp.tile([C, 1], u8)
            nc.scalar.activation(
                warm[:], neg_thresh[:], mybir.ActivationFunctionType.Sign
            )
            TCH = FREE // tail_split
            for _rep in range(reps):
                xts = {}
                for t in range(n_body):
                    xt = xp.tile([C, FREE], f32, tag="xt")
                    nc.sync.dma_start(xt[:], x[t])
                    xts[t] = xt
                tails = {}
                for t in range(n_body, T):
                    for j in range(tail_split):
                        xt = tp.tile([C, TCH], f32, tag="xtail")
                        nc.sync.dma_start(
                            xt[:], x[t, :, j * TCH : (j + 1) * TCH]
                        )
                        tails[(t, j)] = xt
                u_prev = None
                for t in range(T):
                    tail = t >= n_body
                    ns = tail_split if tail else nsplit
                    CHt = FREE // ns
                    if t == 0:
                        u = xts[0]
                    else:
                        ut = up.tile([C, FREE], f32)
                        for j in range(ns):
                            s = slice(j * CHt, (j + 1) * CHt)
                            nc.vector._custom_dve(
                                lif_op,
                                out=ut[:, s],
                                in0=u_prev[:, s],
                                in1=tails[(t, j)][:] if tail else xts[t][:, s],
                                s0=THRESH,
                                s1=TAU,
                            )
                        u = ut
                    yt = yp.tile([C, FREE], u8)
                    for j in range(ns):
                        s = slice(j * CHt, (j + 1) * CHt)
                        nc.scalar.activation(
                            yt[:, s],
                            u[:, s],
                            mybir.ActivationFunctionType.Sign,
                            bias=neg_thresh[:],
                        )
                        nc.scalar.dma_start(y[t, :, s], yt[:, s])
                    u_prev = u
    nc.compile()
    return nc


def build_bass_fused4(reps=1, xp_bufs=None):
    """Natural-layout chunked pipeline: I/O tensors keep the host layout
    [BPC, T, C, HW]; the chunk unit is one batch element, so every load
    (x[b,t], 512KB) and store (y[b,t], 128KB) is fully contiguous in DRAM
    and the host does no transposes at all. Compute is per-chunk: fused LIF
    DVE op + ACT Sign + store right behind it on the ACT ring."""
    import concourse.bacc as bacc
    import concourse.mybir as mybir
    from concourse.tile import TileContext

    lif_op = _register_lif_op()

    f32 = mybir.dt.float32
    u8 = mybir.dt.uint8

    if xp_bufs is None:
        xp_bufs = 2 * T  # half a rep of x chunks in flight

    nc = bacc.Bacc("TRN2", target_bir_lowering=False)
    x = nc.dram_tensor("x", [BPC, T, C, HW], f32, kind="ExternalInput")
    y = nc.dram_tensor("y", [BPC, T, C, HW], u8, kind="ExternalOutput")

    with TileContext(nc) as tc:
        with (
            tc.tile_pool(name="xp", bufs=xp_bufs) as xp,
            tc.tile_pool(name="up", bufs=3) as up,
            tc.tile_pool(name="yp", bufs=3) as yp,
            tc.tile_pool(name="cp", bufs=1) as cp,
        ):
            neg_thresh = cp.tile([C, 1], f32)
            nc.vector.memset(neg_thresh[:], -THRESH)
            warm = cp.tile([C, 1], u8)
            nc.scalar.activation(
                warm[:], neg_thresh[:], mybir.ActivationFunctionType.Sign
            )
            for _rep in range(reps):
                xts = {}
                for t in range(T):
                    for b in range(BPC):
                        xt = xp.tile([C, HW], f32, tag="xt")
                        nc.sync.dma_start(xt[:], x[b, t])
                        xts[(t, b)] = xt
                u_prev = None
                for t in range(T):
                    if t == 0:
                        u = [xts[(0, b)] for b in range(BPC)]
                    else:
                        ut = up.tile([C, BPC, HW], f32)
                        for b in range(BPC):
                            nc.vector._custom_dve(
                                lif_op,
                                out=ut[:, b, :],
                                in0=u_prev[b][:] if t == 1 else u_prev[:, b, :],
                                in1=xts[(t, b)][:],
                                s0=THRESH,
                                s1=TAU,
                            )
                        u = ut
                    yt = yp.tile([C, BPC, HW], u8)
                    for b in range(BPC):
                        nc.scalar.activation(
                            yt[:, b, :],
                            u[b][:] if t == 0 else u[:, b, :],
                            mybir.ActivationFunctionType.Sign,
                            bias=neg_thresh[:],
                        )
                        nc.scalar.dma_start(y[b, t], yt[:, b, :])
                    u_prev = u
    nc.compile()
    return nc


# ---- legacy variant (previous session's kernel) for A/B ---------------------
def build_bass(free_w=HW, use_act=True, reps=1):
    import concourse.bacc as bacc
    import concourse.mybir as mybir
    from concourse.tile import TileContext

    f32 = mybir.dt.float32
    u8 = mybir.dt.uint8
    Alu = mybir.AluOpType

    nc = bacc.Bacc("TRN2", target_bir_lowering=False)
    x = nc.dram_tensor("x", [BPC, T, C, free_w], f32, kind="ExternalInput")
    y = nc.dram_tensor("y", [BPC, T, C, free_w], u8, kind="ExternalOutput")

    with TileContext(nc) as tc:
        with (
            tc.tile_pool(name="xp", bufs=6) as xp,
            tc.tile_pool(name="up", bufs=2) as up,
            tc.tile_pool(name="rp", bufs=2) as rp,
            tc.tile_pool(name="yp", bufs=3) as yp,
            tc.tile_pool(name="cp", bufs=1) as cp,
        ):
            neg_thresh = None
            if use_act:
                neg_thresh = cp.tile([C, 1], f32)
                nc.vector.memset(neg_thresh[:], -THRESH)
            for _rep in range(reps):
                r = None
                for t in range(T):
                    xt = xp.tile([C, BPC, free_w], f32)
                    nc.sync.dma_start(xt[:], x[:, t, :, :].rearrange("b c w -> c b w"))
                    if t == 0:
                        u = xt
                    else:
                        u = up.tile([C, BPC, free_w], f32)
                        nc.vector.scalar_tensor_tensor(
                            u[:], r[:], TAU, xt[:], Alu.mult, Alu.add
                        )
                    yt = yp.tile([C, BPC, free_w], u8)
                    if use_act:
                        nc.scalar.activation(
                            yt[:],
                            u[:],
                            mybir.ActivationFunctionType.Sign,
                            bias=neg_thresh[:],
                        )
                    else:
                        nc.vector.tensor_scalar(yt[:], u[:], THRESH, None, Alu.is_ge)
                    if t < T - 1:
                        rn = rp.tile([C, BPC, free_w], f32)
                        nc.vector.scalar_tensor_tensor(
                            rn[:], u[:], THRESH, u[:], Alu.is_lt, Alu.mult
                        )
                        r = rn
                    nc.scalar.dma_start(
                        y[:, t, :, :].rearrange("b c w -> c b w"), yt[:]
                    )
    nc.compile()
    return nc


# Default device-program config. "nat": natural-layout chunked pipeline —
# chunk unit = one batch element, every DMA fully contiguous, no host
# transposes. Overridable via LIF_CFG for A/B testing; see build_cfg.
DEFAULT_CFG = "nat"


def build_cfg(cfg, reps=1):
    if cfg == "act":
        return build_bass(HW, use_act=True, reps=reps)
    if cfg == "nat":
        return build_bass_fused4(reps=reps)
    if cfg.startswith("ck"):
        ns, ss = cfg[2:].split("s")
        return build_bass_fused2(reps=reps, nsplit=int(ns), store_split=int(ss))
    if cfg.startswith("f3"):
        ns, rest = cfg[3:].split("t")
        ts, tst = (rest.split("s") + ["1"])[:2] if "s" in rest else (rest, "1")
        return build_bass_fused3(
            reps=reps, nsplit=int(ns), tail_split=int(ts), tail_steps=int(tst)
        )
    spl, ns = int(cfg[3]), int(cfg[6])  # spl<k>ns<j>
    return build_bass_fused(reps=reps, steps_per_load=spl, nsplit=ns)


def _get_nc():
    cfg = os.environ.get("LIF_CFG", DEFAULT_CFG)
    if cfg not in _nc_cache:
        _nc_cache[cfg] = build_cfg(cfg)
    return _nc_cache[cfg]


def kernel(x):
    global LAST_RESULTS
    from concourse import bass_utils

    assert x.shape == (B, T, C, H, W) and x.dtype == np.float32
    cfg = os.environ.get("LIF_CFG", DEFAULT_CFG)
    nc = _get_nc()
    if cfg in ("act", "nat"):
        xr = x.reshape(B, T, C, HW)
        in_maps = [{"x": xr[i * BPC : (i + 1) * BPC]} for i in range(N_CORES)]
    else:
        # per core i: x[4i:4i+4] as [T, C, BPC*HW] contiguous
        xs = np.ascontiguousarray(
            x.reshape(N_CORES, BPC, T, C, HW).transpose(0, 2, 3, 1, 4)
        ).reshape(N_CORES, T, C, FREE)
        in_maps = [{"x": xs[i]} for i in range(N_CORES)]
    res = bass_utils.run_bass_kernel_spmd(
        nc,
        in_maps,
        core_ids=list(range(N_CORES)),
        trace=bool(int(os.environ.get("LIF_TRACE", "0"))),
    )
    LAST_RESULTS = res
    out = np.empty((B, T, C, HW), dtype=np.float32)
    for i in range(N_CORES):
        yi = res.results[i]["y"]
        if cfg in ("act", "nat"):
            out[i * BPC : (i + 1) * BPC] = yi == 1
        else:
            # yi [T, C, FREE]; Sign lands {255/0, 0, 1} in uint8; spike==1
            sp = yi.reshape(T, C, BPC, HW).transpose(2, 0, 1, 3)
            out[i * BPC : (i + 1) * BPC] = sp == 1
    return out.reshape(B, T, C, H, W)
